# revision 20
# baseline (speedup 1.0000x reference)
"""Trainium2 Bass kernel for nn_CanadarmJacob (centroidal-dynamics jacobian).

v3: fp16 compute in channel-major layout [P=128 partitions, ch, F samples],
data-parallel over 8 cores.  Host repacks inputs to the 63 used channels
(com 21, link-pos 21, jac rows0:3 21) as fp16; device computes the reduced
graph (validated vs reference in fp32 and fp16, rel ~1.6e-3):

  RP = C - P ; MC = C*m/S ; U = [RP_a*MC_d ; RP*m/M_tot]
  G,R = suffix_j(U) ; trg = tr(G) ; rj = sum_a rpre_a J ; VR = rj_b * R
  rr = sum_a rpre_a R ; u = trg - rr          (R pre-scaled by 1/M_tot)
  hth = (u+DCUM/S)*J - sum_d G*J + VR         (= true H_theta / S)
  jtw = J x R                                 (= true J_tw / M_tot)
  H_s^-1 via 1st-order Neumann (diag-dominant): q = DS/d,
  bot = -g + T2 - T3 with g = q*hth*(S/DS), T2/T3 the off-diag corrections
  top = (r x bot) - jtw

All tensor ops keep the sample dim innermost & packed -> DVE 2x fp16 mode.
front(b) and back(b-1) are generators whose instruction emissions are
interleaved so neither engine's sequencer head-blocks on a long dependent
cluster (wait-queue depth is only 4).
"""

import os
import sys

for _p in ("/opt/trn_rl_repo", "/root/.axon_site/_ro/trn_rl_repo"):
    if os.path.isdir(_p) and _p not in sys.path:
        sys.path.append(_p)

import numpy as np

import concourse.bass as bass
import concourse.tile as tile
from concourse import bacc, mybir
from concourse.bass_utils import run_bass_kernel_spmd

# ----------------------------------------------------------------- constants
N_SAMPLES, N_HORIZON = 2048, 128
N_CORES = 8
P = 128
F = 64  # samples per partition row per block
SPC = N_SAMPLES // N_CORES * N_HORIZON  # samples per core = 32768
NBLK = SPC // (P * F)  # 4

BASE_MASS, EEF_MASS = 100000.0, 243.66
MASS = np.array([105.98, 105.98, 314.98, 279.2, 105.98, 105.98, 243.66], np.float32)
DIAGS = np.array(
    [
        [12.19, 12.19, 3.061],
        [12.19, 12.19, 3.061],
        [15.41, 2094.71, 2103.19],
        [9.522, 1966.28, 1966.28],
        [8.305, 3.061, 8.0386],
        [12.13, 12.13, 3.061],
        [9.336, 44.41, 44.41],
    ],
    np.float32,
)
I0DIAG = np.array([69585.02, 69585.02, 66666.664], np.float32)

M_TOT = float(MASS.sum()) + BASE_MASS + EEF_MASS
K = BASE_MASS + EEF_MASS
BETA = 6.65 * (243.66 / (100000.0 + 243.66))
DCUM = np.stack([DIAGS[j:].sum(0) for j in range(7)], axis=1)  # [a][j]
C1 = DIAGS.sum(0) + I0DIAG  # [a]

S = 64.0   # mass scale for MC/G/hth/...
RS = 16.0  # rE = RS * r
DS = 64.0  # dd = d / DS

DT = mybir.dt.float16
ADD = mybir.AluOpType.add
SUB = mybir.AluOpType.subtract
MUL = mybir.AluOpType.mult

NCCH = 41  # const channels: mp 7, mpm 7, dcum 21, c1 3, bvec 3


def _const_array() -> np.ndarray:
    cst = np.zeros((P, NCCH, F), np.float32)
    cst[:, 0:7, :] = (MASS / S)[None, :, None]
    cst[:, 7:14, :] = (MASS / M_TOT)[None, :, None]
    cst[:, 14:35, :] = (DCUM / S).reshape(21)[None, :, None]
    cst[:, 35:38, :] = (C1 / DS)[None, :, None]
    cst[:, 40, :] = RS * BETA
    return cst.astype(np.float16)


def build_nc():
    nc = bacc.Bacc("TRN2")

    in_d = nc.dram_tensor("inp", [NBLK, P, 63 * F], DT, kind="ExternalInput")
    cst_d = nc.dram_tensor("cst", [P, NCCH * F], DT, kind="ExternalInput")
    out_d = nc.dram_tensor("out", [NBLK, P, 42 * F], DT, kind="ExternalOutput")

    V = nc.vector
    G_ = nc.gpsimd

    with tile.TileContext(nc) as tc:
        with (
            nc.allow_low_precision(reason="fp16 graph validated vs fp32 reference"),
            tc.tile_pool(name="cstp", bufs=1) as cstp,
            tc.tile_pool(name="ioin", bufs=2) as ioin,
            tc.tile_pool(name="ioout", bufs=2) as ioout,
            tc.tile_pool(name="wk", bufs=2) as wk,
        ):
            cst = cstp.tile([P, NCCH * F], DT, tag="cst")
            nc.scalar.dma_start(cst[:], cst_d[:])
            cf = cst[:]
            mp_b = (
                cf[:, 0 : 7 * F].rearrange("p (o x) -> p o x", o=1, x=7 * F)
                .broadcast_to([P, 3, 7 * F])
            )
            mpm_b = (
                cf[:, 7 * F : 14 * F].rearrange("p (o x) -> p o x", o=1, x=7 * F)
                .broadcast_to([P, 3, 7 * F])
            )
            dcum_v = cf[:, 14 * F : 35 * F].rearrange("p (a x) -> p a x", a=3, x=7 * F)
            c1_v = cf[:, 35 * F : 38 * F].rearrange("p (a f) -> p a f", a=3, f=F)
            bvec_f = cf[:, 38 * F : 41 * F]  # [P, 3F] = (0,0,RS*BETA)
            zero_b = (
                cf[:, 38 * F : 39 * F].rearrange("p (o f) -> p o f", o=1, f=F)
                .broadcast_to([P, 3, F])
            )

            def t(ch, tag):
                return wk.tile([P, ch * F], DT, tag=tag, name=tag)[:]

            def front(b, st):
                int_ = ioin.tile([P, 63 * F], DT, tag="int")
                nc.sync.dma_start(int_[:], in_d[b])
                yield
                iv = int_[:]
                com_f = iv[:, 0 : 21 * F]
                pos_f = iv[:, 21 * F : 42 * F]
                jac_f = iv[:, 42 * F : 63 * F]
                com3 = com_f.rearrange("p (a x) -> p a x", a=3, x=7 * F)
                jac3 = jac_f.rearrange("p (a x) -> p a x", a=3, x=7 * F)
                jac4 = jac_f.rearrange("p (a j f) -> p a j f", a=3, j=7, f=F)
                st["jac3"], st["jac4"], st["jac_f"] = jac3, jac4, jac_f

                rp = t(21, "rp")
                V.tensor_sub(rp, com_f, pos_f)
                yield
                rp3 = rp.rearrange("p (a x) -> p a x", a=3, x=7 * F)
                mc = t(21, "mc")
                mc3 = mc.rearrange("p (a x) -> p a x", a=3, x=7 * F)
                V.tensor_mul(mc3, com3, mp_b)
                yield

                ut = t(84, "ut")
                u4 = ut.rearrange("p (a d x) -> p a d x", a=3, d=4, x=7 * F)
                rp_b = rp3.unsqueeze(2).broadcast_to([P, 3, 3, 7 * F])
                mc_b = mc3.unsqueeze(1).broadcast_to([P, 3, 3, 7 * F])
                V.tensor_mul(u4[:, :, 0:3], rp_b, mc_b)
                yield
                V.tensor_mul(u4[:, :, 3], rp3, mpm_b)
                yield

                mc_ai = mc.rearrange("p (a i f) -> p a i f", a=3, i=7, f=F)
                y9 = t(9, "y9").rearrange("p (a c f) -> p a c f", a=3, c=3, f=F)
                V.tensor_add(y9, mc_ai[:, :, 0:3], mc_ai[:, :, 3:6])
                yield
                rpre = t(3, "rpre").rearrange("p (a f) -> p a f", a=3, f=F)
                V.tensor_add(rpre, y9[:, :, 0], y9[:, :, 1])
                yield
                V.tensor_add(rpre, rpre, y9[:, :, 2])
                yield
                V.tensor_add(rpre, rpre, mc_ai[:, :, 6])
                yield
                st["rpre"] = rpre

                gt = ut.rearrange("p (c j f) -> p c j f", c=12, j=7, f=F)
                for j in range(5, -1, -1):
                    V.tensor_add(gt[:, :, j], gt[:, :, j], gt[:, :, j + 1])
                    yield
                st["u4"] = u4
                rsuf3 = u4[:, :, 3]
                st["rsuf3"] = rsuf3

                trg = t(7, "trg")
                V.tensor_add(trg, u4[:, 0, 0], u4[:, 1, 1])
                yield
                V.tensor_add(trg, trg, u4[:, 2, 2])
                yield
                st["trg"] = trg

                rjp = t(21, "rjp")
                rjp4 = rjp.rearrange("p (a j f) -> p a j f", a=3, j=7, f=F)
                rpre_bj = rpre.unsqueeze(2).broadcast_to([P, 3, 7, F])
                V.tensor_mul(rjp4, rpre_bj, jac4)
                yield
                rj = t(7, "rj")
                G_.tensor_add(rj, rjp[:, 0 : 7 * F], rjp[:, 7 * F : 14 * F])
                yield
                G_.tensor_add(rj, rj, rjp[:, 14 * F : 21 * F])
                yield

                vr = t(21, "vr")
                vr3 = vr.rearrange("p (a x) -> p a x", a=3, x=7 * F)
                rj_b = (
                    rj.rearrange("p (o x) -> p o x", o=1, x=7 * F)
                    .broadcast_to([P, 3, 7 * F])
                )
                G_.tensor_mul(vr3, rj_b, rsuf3)
                yield
                st["vr"] = vr

                rrp = t(21, "rrp")
                rrp4 = rrp.rearrange("p (a j f) -> p a j f", a=3, j=7, f=F)
                rsuf4 = u4[:, :, 3].rearrange("p a (j f) -> p a j f", j=7, f=F)
                G_.tensor_mul(rrp4, rpre_bj, rsuf4)
                yield
                rr = t(7, "rr")
                G_.tensor_add(rr, rrp[:, 0 : 7 * F], rrp[:, 7 * F : 14 * F])
                yield
                G_.tensor_add(rr, rr, rrp[:, 14 * F : 21 * F])
                yield
                st["rr"] = rr

            def back(st, b):
                jac3, jac4, jac_f = st["jac3"], st["jac4"], st["jac_f"]
                u4, rsuf3 = st["u4"], st["rsuf3"]
                rpre, trg, rr = st["rpre"], st["trg"], st["rr"]
                vr = st["vr"]

                outt = ioout.tile([P, 42 * F], DT, tag="outt")
                ov = outt[:]
                top_f = ov[:, 0 : 21 * F]
                bot_f = ov[:, 21 * F : 42 * F]

                # jtw = J x R (Pool; independent of the bot chain)
                jtw = t(21, "jtw")
                cx1 = t(21, "cx1")
                jtw3 = jtw.rearrange("p (a x) -> p a x", a=3, x=7 * F)
                cx13 = cx1.rearrange("p (a x) -> p a x", a=3, x=7 * F)
                for a in range(3):
                    a1_, a2_ = (a + 1) % 3, (a + 2) % 3
                    G_.tensor_mul(jtw3[:, a], jac3[:, a1_], rsuf3[:, a2_])
                    yield
                    G_.tensor_mul(cx13[:, a], jac3[:, a2_], rsuf3[:, a1_])
                    yield
                G_.tensor_sub(jtw, jtw, cx1)
                yield

                u7 = t(7, "u7")
                V.tensor_sub(u7, trg, rr)
                yield
                a1 = t(21, "a1")
                a1_3 = a1.rearrange("p (a x) -> p a x", a=3, x=7 * F)
                u_b = (
                    u7.rearrange("p (o x) -> p o x", o=1, x=7 * F)
                    .broadcast_to([P, 3, 7 * F])
                )
                V.tensor_add(a1_3, u_b, dcum_v)
                yield

                gd = u4[:, :, 0:3]
                jac_bd = jac3.unsqueeze(1).broadcast_to([P, 3, 3, 7 * F])
                V.tensor_mul(gd, gd, jac_bd)
                yield
                t1 = t(21, "t1")
                t1_3 = t1.rearrange("p (a x) -> p a x", a=3, x=7 * F)
                V.tensor_add(t1_3, u4[:, :, 0], u4[:, :, 1])
                yield
                V.tensor_add(t1_3, t1_3, u4[:, :, 2])
                yield

                hth = t(21, "hth")
                V.tensor_mul(hth, a1, jac_f)
                yield
                V.tensor_sub(hth, hth, t1)
                yield
                V.tensor_add(hth, hth, vr)
                yield
                hth4 = hth.rearrange("p (a j f) -> p a j f", a=3, j=7, f=F)

                rpre_f = rpre.rearrange("p a f -> p (a f)")
                rE = t(3, "rE")
                V.scalar_tensor_tensor(rE, rpre_f, RS * S / M_TOT, bvec_f, MUL, SUB)
                yield
                rE3 = rE.rearrange("p (a f) -> p a f", a=3, f=F)
                p2 = t(3, "p2")
                V.tensor_mul(p2, rE, rE)
                yield
                s2 = t(1, "s2")
                V.tensor_add(s2, p2[:, 0:F], p2[:, F : 2 * F])
                yield
                V.tensor_add(s2, s2, p2[:, 2 * F : 3 * F])
                yield
                # D' = diag(C1 - K|r|^2); E' = K r r^T (full outer product)
                KS = float(K / (RS * RS * DS))
                t3 = t(3, "t3")
                t3_3 = t3.rearrange("p (a f) -> p a f", a=3, f=F)
                s2_b = (
                    s2.rearrange("p (o f) -> p o f", o=1, f=F).broadcast_to([P, 3, F])
                )
                V.scalar_tensor_tensor(t3_3, s2_b, -KS, c1_v, MUL, ADD)
                yield
                qq = t(3, "qq")
                V.reciprocal(qq, t3)
                yield
                qq3 = qq.rearrange("p (a f) -> p a f", a=3, f=F)

                g = t(21, "g")
                g4 = g.rearrange("p (a j f) -> p a j f", a=3, j=7, f=F)
                qq_b = qq3.unsqueeze(2).broadcast_to([P, 3, 7, F])
                V.tensor_mul(g4, qq_b, hth4)
                yield
                u1 = t(21, "u1")
                u1_4 = u1.rearrange("p (a j f) -> p a j f", a=3, j=7, f=F)
                rE_b = rE3.unsqueeze(2).broadcast_to([P, 3, 7, F])
                V.tensor_mul(u1_4, rE_b, g4)
                yield
                w7 = t(7, "w7")
                V.tensor_add(w7, u1[:, 0 : 7 * F], u1[:, 7 * F : 14 * F])
                yield
                V.tensor_add(w7, w7, u1[:, 14 * F : 21 * F])
                yield
                v1 = t(3, "v1")
                V.scalar_tensor_tensor(v1, qq, KS, rE, MUL, MUL)
                yield
                z1 = t(21, "z1")
                z1_4 = z1.rearrange("p (a j f) -> p a j f", a=3, j=7, f=F)
                v1_b = (
                    v1.rearrange("p (a f) -> p a f", a=3, f=F)
                    .unsqueeze(2)
                    .broadcast_to([P, 3, 7, F])
                )
                w_b = (
                    w7.rearrange("p (j f) -> p j f", j=7, f=F)
                    .unsqueeze(1)
                    .broadcast_to([P, 3, 7, F])
                )
                V.tensor_mul(z1_4, v1_b, w_b)
                yield
                V.tensor_sub(bot_f, z1, g)
                yield

                rQ = t(3, "rQ")
                rQ3 = rQ.rearrange("p (a f) -> p a f", a=3, f=F)
                V.scalar_tensor_tensor(rQ3, rE3, 1.0 / RS, zero_b, MUL, ADD)
                yield
                ctb = t(21, "ctb")
                ctc = t(21, "ctc")
                ctb3 = ctb.rearrange("p (a x) -> p a x", a=3, x=7 * F)
                ctc3 = ctc.rearrange("p (a x) -> p a x", a=3, x=7 * F)
                bot3 = bot_f.rearrange("p (a x) -> p a x", a=3, x=7 * F)
                for a in range(3):
                    a1_, a2_ = (a + 1) % 3, (a + 2) % 3
                    r1 = rQ3[:, a1_ : a1_ + 1, :].broadcast_to([P, 7, F])
                    r2 = rQ3[:, a2_ : a2_ + 1, :].broadcast_to([P, 7, F])
                    b2 = bot3[:, a2_].rearrange("p (j f) -> p j f", j=7, f=F)
                    b1 = bot3[:, a1_].rearrange("p (j f) -> p j f", j=7, f=F)
                    G_.tensor_mul(
                        ctb3[:, a].rearrange("p (j f) -> p j f", j=7, f=F), r1, b2
                    )
                    yield
                    G_.tensor_mul(
                        ctc3[:, a].rearrange("p (j f) -> p j f", j=7, f=F), r2, b1
                    )
                    yield
                V.tensor_sub(ctb, ctb, ctc)
                yield
                V.tensor_sub(top_f, ctb, jtw)
                yield

                nc.scalar.dma_start(out_d[b], outt[:])
                yield

            def drain(gen):
                for _ in gen:
                    pass

            def interleave(bg, fg, nb=2, nf=1):
                done_b = done_f = False
                while not (done_b and done_f):
                    for _ in range(nb):
                        if not done_b:
                            try:
                                next(bg)
                            except StopIteration:
                                done_b = True
                    for _ in range(nf):
                        if not done_f:
                            try:
                                next(fg)
                            except StopIteration:
                                done_f = True

            st_prev = None
            for b in range(NBLK):
                st = {}
                fg = front(b, st)
                if st_prev is None:
                    drain(fg)
                else:
                    interleave(back(st_prev, b - 1), fg)
                st_prev = st
            drain(back(st_prev, NBLK - 1))

    nc.compile()
    return nc


_NC_CACHE = None


def _get_nc():
    global _NC_CACHE
    if _NC_CACHE is None:
        _NC_CACHE = build_nc()
    return _NC_CACHE


def _shard_inputs(com_list, link_pose_list, jacobian):
    com = np.asarray(com_list, np.float32).reshape(N_SAMPLES * N_HORIZON, 3, 7)
    lnk = np.asarray(link_pose_list, np.float32).reshape(
        N_SAMPLES * N_HORIZON, 4, 4, 9
    )
    jac = np.asarray(jacobian, np.float32).reshape(N_SAMPLES * N_HORIZON, 6, 7)
    pos = lnk[:, :3, 3, :7]  # (n,3,7)
    j3 = jac[:, :3, :]  # (n,3,7)

    # pack channels: [com 21, pos 21, jac 21] -> fp16 ch-major [NBLK,P,63,F]
    packed = np.concatenate(
        [com.reshape(-1, 21), pos.reshape(-1, 21), j3.reshape(-1, 21)], axis=1
    ).astype(np.float16)  # (n, 63)

    cst = _const_array().reshape(P, NCCH * F)
    in_maps = []
    for c in range(N_CORES):
        blk = packed[c * SPC : (c + 1) * SPC].reshape(NBLK, P, F, 63)
        blk = np.ascontiguousarray(blk.transpose(0, 1, 3, 2))  # [NBLK,P,63,F]
        in_maps.append({"inp": blk.reshape(NBLK, P, 63 * F), "cst": cst})
    return in_maps


def _gather(results):
    outs = []
    for r in results:
        o = np.asarray(r["out"], np.float32).reshape(NBLK, P, 42, F)
        o = o.transpose(0, 1, 3, 2).reshape(SPC, 6, 7)
        outs.append(o)
    full = np.concatenate(outs, axis=0)
    return np.ascontiguousarray(full.reshape(N_SAMPLES, N_HORIZON, 6, 7))


def run(com_list, link_pose_list, jacobian, trace=False):
    nc = _get_nc()
    in_maps = _shard_inputs(com_list, link_pose_list, jacobian)
    res = run_bass_kernel_spmd(nc, in_maps, list(range(N_CORES)), trace=trace)
    return _gather(res.results), res


def kernel(com_list, link_pose_list, jacobian):
    out, _ = run(com_list, link_pose_list, jacobian)
    return out
